# revision 29
# baseline (speedup 1.0000x reference)
"""Trainium2 Bass kernel for nn_AttentionHead_Hybrid2 (B=4, N=4096, DK=64).

reference:
    V = x @ Wv.T + bv              (B,N,DK)
    Q = x @ wq ; K = x @ wk        (B,N)
    A = exp(-(Q_i - K_j)^2)        (B,N,N)
    P = softmax(A / 8, axis=-1)
    out = LN(P @ V + x)

Sharding: 8 cores = (batch b = c//2) x (query half c%2). Each core gets the
full key set for its batch (rolled so its 2048 queries are rows 0:2048) and
produces its 2048x64 output slice.

Algorithm (Fourier separation): the score E(q,k) = exp(exp(-(q-k)^2)/8)
depends only on t = q - k, so it has a rapidly-converging cosine expansion
E(t) = sum_k a_k cos(w_k t) (periodized, L=13, 24 cos/sin features gives
~3e-5 abs accuracy). cos(w(Q-K)) = cosQcosK + sinQsinK makes attention
separable with rank 24:
    num (2048, 66) = PhiQ (2048,24) @ [ a*(Wv-transformed PhiK-moments) ]
where PhiK/PhiQ are sin/cos feature maps of the key/query scalar
projections; col 64 = softmax denominator, col 65 = numerator row-sum
(for LN stats). No (N,N) scores, no binning, no big exp fields.

Phases are computed in turns r = u/2pi directly from x: per 128-token tile,
u_tile = xth_tile.T @ W2 with W2 = w (x) k/L + phase-row (bf16 hi/lo split;
residual phase errors are incoherent across keys and wash out). The ACT sin
table is only valid on [-pi,pi], so one DVE magic-number round pass forms
w = r - round(r) in [-0.5,0.5] and the ACT evaluates sin(2pi*w).

Query features are computed tile-major like keys (sharing the stationary
xth tile), then PE-transposed (bf16) to features-on-partitions so each
numerator matmul lands tokens-on-partitions - the LN tail needs no
transposes at all.

LayerNorm is scale-invariant, so no division by the softmax denominator:
z = num + den*x, out = (z - mean) * rsqrt(E[z^2] - mean^2); gamma/beta are
ones/zeros per the problem spec (host applies them if they ever are not).
Ln/Exp are batched once so the ACT table (trig <-> ln/exp set) switches
exactly once mid-kernel.
"""

import math
import sys

for _p in ("/opt/trn_rl_repo", "/root/.axon_site/_ro/trn_rl_repo"):
    if _p not in sys.path:
        sys.path.insert(0, _p)

import numpy as np

import concourse.bass as bass
import concourse.mybir as mybir
import concourse.tile as tile
import bass_rust
from concourse.bass_utils import run_bass_kernel_spmd

F32 = mybir.dt.float32
BF16 = mybir.dt.bfloat16
AF = mybir.ActivationFunctionType
OP = mybir.AluOpType

B, N, DK = 4, 4096, 64
NQ = 2048          # queries per core
NCORES = 8
JT = N // 128      # 32 key tiles
IT = NQ // 128     # 16 query tiles
LPER = 13.0        # Fourier period in t = q - k
NF = 24            # features: cos k=0..12, sin k=1..11
MAGIC = float(np.float32(1.5 * 2 ** 23))   # fp32 round-to-nearest trick
GRP = 16           # phase tiles per round/sin group

# const blob column layout (fp32)
_WVB0 = 0                 # (65, 66)
_ACOL = _WVB0 + 66        # (24, 1)  feature coefficients
BLOB_W = _ACOL + 1


def split_multiwaits(nc):
    """Walrus in this env accepts one sem-wait per instruction; Tile emits
    several. Split extras onto preceding same-engine NoOps."""
    ctr = 0
    for f in nc.m.functions:
        for bb in f.blocks:
            out, changed = [], False
            for ins in bb.instructions:
                si = ins.sync_info
                if si is not None and si.on_wait and len(si.on_wait) > 1:
                    waits = list(si.on_wait)
                    for w in waits[:-1]:
                        ctr += 1
                        out.append(mybir.InstNoOp(
                            name=f"I-wsplit-{ctr}", engine=ins.engine,
                            debug=ins.debug, ins=[], outs=[],
                            sync_info=bass_rust.SyncInfo(on_wait=[w], on_update=[])))
                    ins.sync_info = bass_rust.SyncInfo(
                        on_wait=[waits[-1]], on_update=list(si.on_update or []))
                    changed = True
                out.append(ins)
            if changed:
                bb.instructions = out
    return ctr


def build_nc(split=True):
    nc = bass.Bass("TRN2", target_bir_lowering=False, debug=False)

    blob_d = nc.dram_tensor("blob", [128, BLOB_W], F32, kind="ExternalInput").ap()
    wf_d = nc.dram_tensor("wf", [65, 4 * NF + 1], BF16, kind="ExternalInput").ap()
    idb_d = nc.dram_tensor("idb", [128, 128], BF16, kind="ExternalInput").ap()
    xth_d = nc.dram_tensor("xth", [DK + 1, N], BF16, kind="ExternalInput").ap()
    xa_d = nc.dram_tensor("xa", [128, JT * 65], BF16, kind="ExternalInput").ap()
    xl_d = nc.dram_tensor("xl", [128, IT * DK], BF16, kind="ExternalInput").ap()
    out_d = nc.dram_tensor("out", [128, IT * DK], F32, kind="ExternalOutput").ap()

    with tile.TileContext(nc) as tc:
        cpool = tc.alloc_tile_pool(name="consts", bufs=1)
        big = tc.alloc_tile_pool(name="big", bufs=1)

        blob = cpool.tile([128, BLOB_W], F32)
        nc.sync.dma_start(blob[:], blob_d[:])
        wf = cpool.tile([65, 4 * NF + 1], BF16)
        nc.sync.dma_start(wf[:], wf_d[:])
        wvb = blob[0:65, _WVB0:_WVB0 + 66]
        acol = blob[0:24, _ACOL:_ACOL + 1]
        w2kh = wf[:, 0 * NF:1 * NF]
        w2kl = wf[:, 1 * NF:2 * NF]
        w2qh = wf[:, 2 * NF:3 * NF]
        w2ql = wf[:, 3 * NF:4 * NF]
        csel = wf[:, 4 * NF:4 * NF + 1]          # [1]*64 + [0]: row-sum select

        xth = big.tile([DK + 1, N], BF16)
        for h in (4, 5, 6, 7, 0, 1, 2, 3):      # tiles 16..31 are consumed first
            nc.sync.dma_start(xth[:, h * 512:(h + 1) * 512],
                              xth_d[:, h * 512:(h + 1) * 512])

        idb = cpool.tile([128, 128], BF16)
        nc.sync.dma_start(idb[:], idb_d[:])

        # host-pretiled natural layouts: [p, tile, col]; split into many
        # dma_starts - each lands on one HW queue at ~22 GB/s, so spreading
        # is what buys aggregate bandwidth
        xa_all = big.tile([128, JT * 65], BF16)
        xa_v = xa_all.rearrange("p (t c) -> p t c", c=65)
        xl_all = big.tile([128, IT * DK], BF16)
        xl_v = xl_all.rearrange("p (t c) -> p t c", c=DK)
        for h in range(8):
            nc.sync.dma_start(xa_all[:, h * 260:(h + 1) * 260],
                              xa_d[:, h * 260:(h + 1) * 260])
        for h in range(4):
            nc.sync.dma_start(xl_all[:, h * 256:(h + 1) * 256],
                              xl_d[:, h * 256:(h + 1) * 256])

        # ---- phase features ----
        # groups: 0 = keys 16..31, 1 = keys 0..15, 2 = queries (= tiles 0..15)
        phk = big.tile([128, JT * NF], BF16)      # key features, tile-major
        phk_v = phk.rearrange("p (t f) -> p t f", f=NF)
        phqt = big.tile([128, IT * NF], BF16)     # query features, tile-major
        phqt_v = phqt.rearrange("p (t f) -> p t f", f=NF)
        phq = big.tile([24, NQ], BF16)            # query features, rotated
        w_sb = big.tile([128, 3 * GRP * NF], F32)  # reduced phases w = r - rt
        w_v = w_sb.rearrange("p (g c) -> p g c", c=GRP * NF)

        with (tc.tile_pool(name="u_ps", bufs=2, space="PSUM") as ups,
              tc.tile_pool(name="u2_ps", bufs=1, space="PSUM") as u2ps,
              tc.tile_pool(name="pq_ps", bufs=4, space="PSUM") as pqps,
              tc.tile_pool(name="facc_ps", bufs=1, space="PSUM") as faccp,
              tc.tile_pool(name="rt_sb", bufs=3) as rtp):
            u1 = ups.tile([128, GRP * NF], F32, tag="u")
            u1_t = u1.rearrange("p (t f) -> p t f", f=NF)
            for i in range(GRP):                  # keys 16..31 first
                sl = xth[:, (GRP + i) * 128:(GRP + i + 1) * 128]
                nc.tensor.matmul(u1_t[:, i, :], sl, w2kh, start=True, stop=False)
                nc.tensor.matmul(u1_t[:, i, :], sl, w2kl, start=False, stop=True)
            u0 = ups.tile([128, GRP * NF], F32, tag="u")
            u0_t = u0.rearrange("p (t f) -> p t f", f=NF)
            u2 = u2ps.tile([128, GRP * NF + GRP], F32, tag="u2")
            u2_t = u2.rearrange("p c -> p c")
            u2f = u2[:, 0:GRP * NF].rearrange("p (t f) -> p t f", f=NF)
            for i in range(GRP):                  # keys 0..15 + queries + xsum
                sl = xth[:, i * 128:(i + 1) * 128]
                nc.tensor.matmul(u0_t[:, i, :], sl, w2kh, start=True, stop=False)
                nc.tensor.matmul(u0_t[:, i, :], sl, w2kl, start=False, stop=True)
                nc.tensor.matmul(u2f[:, i, :], sl, w2qh, start=True, stop=False)
                nc.tensor.matmul(u2f[:, i, :], sl, w2ql, start=False, stop=True)
                nc.tensor.matmul(u2[:, GRP * NF + i:GRP * NF + i + 1], sl,
                                 csel, start=True, stop=True)

            fwh = big.tile([24, 66], BF16)
            fwl = big.tile([24, 66], BF16)
            f_sb = big.tile([65, NF], F32)
            f_ps = faccp.tile([65, NF], F32, tag="f")
            dummy = big.tile([1, 1], F32)

            for g, u in ((0, u1[:]), (1, u0[:]), (2, u2[:, 0:GRP * NF])):
                rt = rtp.tile([128, GRP * NF], F32, tag="rt")
                nc.vector.tensor_scalar(rt[:], u, MAGIC, MAGIC, OP.add,
                                        OP.subtract)
                nc.vector.tensor_tensor(w_v[:, g, :], u, rt[:], OP.subtract)
                if g == 0:
                    dst = phk[:, GRP * NF:2 * GRP * NF]
                elif g == 1:
                    dst = phk[:, 0:GRP * NF]
                else:
                    dst = phqt[:]
                nc.scalar.activation(dst, w_v[:, g, :], AF.Sin,
                                     scale=2 * math.pi)
                if g == 0:
                    # key moments for tiles 16..31 while queries still cook
                    for jt in range(GRP, JT):
                        nc.tensor.matmul(f_ps[:], xa_v[:, jt, :],
                                         phk_v[:, jt, :],
                                         start=(jt == GRP), stop=False)

            # stage the x row sums to SBUF while u2 is still live
            xsum = big.tile([128, IT], F32)
            nc.vector.tensor_copy(xsum[:], u2[:, GRP * NF:GRP * NF + GRP])
            # trigger the trig -> ln/exp ACT table switch off the critical
            # path; input depends on the last Sin so it cannot be hoisted
            nc.scalar.activation(dummy[:], phqt[0:1, 0:1], AF.Ln, scale=1.0)

            # transpose query features to (24, NQ), bf16
            for q in range(4):
                pt = pqps.tile([24, 512], BF16, tag="pt")
                for i in range(4):
                    nc.tensor.transpose(pt[:, i * 128:(i + 1) * 128],
                                        phqt_v[:, q * 4 + i, :], idb)
                nc.vector.tensor_copy(phq[:, q * 512:(q + 1) * 512], pt[:])

            for jt in range(GRP):
                nc.tensor.matmul(f_ps[:], xa_v[:, jt, :], phk_v[:, jt, :],
                                 start=False, stop=(jt == GRP - 1))
            nc.vector.tensor_copy(f_sb[:], f_ps[:])

        # ---- Fw = (F.T @ WVB) * a, split hi/lo bf16 ----
        with tc.tile_pool(name="fw_ps", bufs=1, space="PSUM") as fps:
            fw_ps = fps.tile([24, 66], F32, tag="fw")
            nc.tensor.matmul(fw_ps[:], f_sb[:], wvb, start=True, stop=True)
            fwm = big.tile([24, 66], F32)
            nc.vector.tensor_tensor(fwm[:], fw_ps[:],
                                    acol.broadcast_to([24, 66]), OP.mult)
            nc.vector.tensor_copy(fwh[:], fwm[:])
            nc.vector.tensor_tensor(fwl[:], fwm[:], fwh[:], OP.subtract)

        # residual x (fp32), off the critical path on gpsimd
        xq = big.tile([128, IT * DK], F32)
        xq_v = xq.rearrange("p (t d) -> p t d", d=DK)
        nc.gpsimd.tensor_tensor(xq_v[:], xa_v[:, 0:IT, 0:DK], xl_v[:], OP.add)

        # ---- numerator (tokens on partitions) + fused LN tail ----
        z_sb = big.tile([128, IT * DK], F32)
        z_v = z_sb.rearrange("p (t d) -> p t d", d=DK)
        o_sb = big.tile([128, IT * DK], F32)
        o_v = o_sb.rearrange("p (t d) -> p t d", d=DK)
        sq = big.tile([128, 8 * DK], F32)
        sq_v = sq.rearrange("p (t d) -> p t d", d=DK)
        st = big.tile([128, 8 * IT], F32)   # stats: s2, mu, var, rstd, -mu*rstd
        s2 = st[:, 0 * IT:1 * IT]
        mu_c = st[:, 2 * IT:3 * IT]
        var_c = st[:, 3 * IT:4 * IT]
        rstd_c = st[:, 4 * IT:5 * IT]
        nmu_c = st[:, 5 * IT:6 * IT]
        t1 = big.tile([128, 8 * DK], F32)
        t1_v = t1.rearrange("p (t d) -> p t d", d=DK)

        with tc.tile_pool(name="num_ps", bufs=4, space="PSUM") as nps:
            nvs = []
            for h in range(4):
                nf = nps.tile([128, 4 * 128], F32, tag="nf")
                nf_v = nf.rearrange("p (t c) -> p t c", c=128)
                nvs.append(nf_v)
                for i in range(4):
                    it = h * 4 + i
                    lhs = phq[:, it * 128:(it + 1) * 128]
                    nc.tensor.matmul(nf_v[:, i, 0:66], lhs, fwh[:],
                                     start=True, stop=False)
                    nc.tensor.matmul(nf_v[:, i, 0:66], lhs, fwl[:],
                                     start=False, stop=True)
            for h in range(4):
                ts_, te_ = h * 4, (h + 1) * 4
                sl = slice((h % 2) * 4, (h % 2) * 4 + 4)
                nv = nvs[h]
                mu3 = mu_c[:, ts_:te_].unsqueeze(-1)
                # z = num + den * x
                nc.vector.tensor_tensor(
                    t1_v[:, sl, :], xq_v[:, ts_:te_, :],
                    nv[:, :, 64:65].broadcast_to([128, 4, DK]), OP.mult)
                nc.vector.tensor_tensor(z_v[:, ts_:te_, :], t1_v[:, sl, :],
                                        nv[:, :, 0:64], OP.add)
                # sum z^2 (Square is in every ACT table set: no switch)
                nc.scalar.activation(sq_v[:, sl, :].rearrange("p t d -> p (t d)"),
                                     z_v[:, ts_:te_, :].rearrange("p t d -> p (t d)"),
                                     AF.Square, scale=1.0)
                nc.vector.reduce_sum(s2[:, ts_:te_], sq_v[:, sl, :],
                                     axis=mybir.AxisListType.X)
                # mu*64 = numsum + den*xsum
                nc.vector.tensor_tensor(mu3, nv[:, :, 64:65],
                                        xsum[:, ts_:te_].unsqueeze(-1), OP.mult)
                nc.vector.tensor_tensor(mu3, mu3, nv[:, :, 65:66], OP.add)
            # batched stats: mu, var = s2/64 - mu^2, rstd = exp(-.5 ln var)
            nc.vector.tensor_scalar_mul(mu_c[:], mu_c[:], 1.0 / DK)
            nc.vector.tensor_tensor(var_c[:], mu_c[:], mu_c[:], OP.mult)
            nc.vector.scalar_tensor_tensor(var_c[:], s2[:], 1.0 / DK, var_c[:],
                                           OP.mult, OP.subtract)
            nc.scalar.activation(rstd_c[:], var_c[:], AF.Ln, scale=1.0)
            nc.scalar.activation(rstd_c[:], rstd_c[:], AF.Exp, scale=-0.5)
            nc.vector.tensor_tensor(nmu_c[:], mu_c[:], rstd_c[:], OP.mult)
            nc.vector.tensor_scalar_mul(nmu_c[:], nmu_c[:], -1.0)
            # out = z*rstd - mu*rstd; chunks 2,3 on the DVE, 0,1 on the ACT
            for h in (2, 3):
                ts_, te_ = h * 4, (h + 1) * 4
                nc.vector.tensor_tensor(
                    o_v[:, ts_:te_, :], z_v[:, ts_:te_, :],
                    rstd_c[:, ts_:te_].unsqueeze(-1).broadcast_to([128, 4, DK]),
                    OP.mult)
                nc.vector.tensor_tensor(
                    o_v[:, ts_:te_, :], o_v[:, ts_:te_, :],
                    nmu_c[:, ts_:te_].unsqueeze(-1).broadcast_to([128, 4, DK]),
                    OP.add)
                for q in range(2):
                    c0 = (ts_ + 2 * q) * DK
                    nc.sync.dma_start(out_d[:, c0:c0 + 2 * DK],
                                      o_sb[:, c0:c0 + 2 * DK])
            for it in range(8):
                nc.scalar.activation(o_v[:, it, :], z_v[:, it, :], AF.Identity,
                                     bias=nmu_c[:, it:it + 1],
                                     scale=rstd_c[:, it:it + 1])
                if it % 2 == 1:
                    c0 = (it - 1) * DK
                    nc.sync.dma_start(out_d[:, c0:c0 + 2 * DK],
                                      o_sb[:, c0:c0 + 2 * DK])

        big.release()
        cpool.release()

    if split:
        split_multiwaits(nc)
    return nc


_NC_CACHE = None


def _get_nc():
    global _NC_CACHE
    if _NC_CACHE is None:
        _NC_CACHE = build_nc()
    return _NC_CACHE


def _fourier_coeffs():
    m = 16384
    t = LPER * np.arange(m) / m
    tw = np.minimum(t, LPER - t)
    g = np.exp(np.exp(-tw ** 2) / 8.0) - 1.0
    c = np.fft.rfft(g) / m
    a_cos = np.concatenate([[1.0 + np.real(c[0])], 2 * np.real(c[1:13])])
    a_sin = 2 * np.real(c[1:12])
    return np.concatenate([a_cos, a_sin]).astype(np.float32)


def make_in_maps(x, Wv, bv, wq, wk, gamma, beta):
    import ml_dtypes
    bf = ml_dtypes.bfloat16
    x = np.asarray(x, np.float32)
    kfeat = np.concatenate([np.arange(13), np.arange(1, 12)]).astype(np.float64)
    phip = np.concatenate([0.25 * np.ones(13), np.zeros(11)])

    wvb = np.zeros((65, 66), np.float32)
    wvb[:64, :64] = np.asarray(Wv, np.float32).T
    wvb[64, :64] = np.asarray(bv, np.float32)
    wvb[64, 64] = 1.0
    wvb[:, 65] = wvb[:, :64].sum(1)

    blob = np.zeros((128, BLOB_W), np.float32)
    blob[0:65, _WVB0:_WVB0 + 66] = wvb
    blob[0:24, _ACOL] = _fourier_coeffs()

    def w2pair(w):
        full = np.concatenate(
            [np.outer(np.asarray(w, np.float64), kfeat / LPER),
             phip[None, :]], 0).astype(np.float32)
        hi = full.astype(bf)
        lo = (full - hi.astype(np.float32)).astype(bf)
        return hi, lo

    wkh, wkl = w2pair(wk)
    wqh, wql = w2pair(wq)
    csel = np.concatenate([np.ones(64, np.float32), [0.0]])[:, None]
    wf = np.concatenate([wkh, wkl, wqh, wql, csel.astype(bf)], 1).astype(bf)
    idb = np.eye(128, dtype=bf)

    ones = np.ones((N, 1), np.float32)
    in_maps = []
    for c in range(NCORES):
        b, qoff = c // 2, (c % 2) * NQ
        xr = np.concatenate([x[b, qoff:], x[b, :qoff]], axis=0) if qoff else x[b]
        xth = np.concatenate([xr.T, ones.T], 0).astype(bf)
        xaf = np.concatenate([xr, ones], 1)
        xa = xaf.astype(bf)                                  # (N, 65)
        xl = (xr[0:NQ] - xa[0:NQ, 0:DK].astype(np.float32)).astype(bf)
        # pre-tile to [p, tile, col] so device DMAs are contiguous
        xa_t = np.ascontiguousarray(
            xa.reshape(JT, 128, 65).transpose(1, 0, 2).reshape(128, JT * 65))
        xl_t = np.ascontiguousarray(
            xl.reshape(IT, 128, DK).transpose(1, 0, 2).reshape(128, IT * DK))
        in_maps.append({"xth": np.ascontiguousarray(xth),
                        "xa": xa_t, "xl": xl_t,
                        "wf": wf, "idb": idb, "blob": blob})
    return in_maps


def kernel(x, Wv, bv, wq, wk, gamma, beta, _trace=False, _trace_cores=None):
    nc = _get_nc()
    in_maps = make_in_maps(x, Wv, bv, wq, wk, gamma, beta)
    res = run_bass_kernel_spmd(nc, in_maps, core_ids=list(range(NCORES)),
                               trace=_trace, trace_cores=_trace_cores)
    out = np.empty((B, N, DK), np.float32)
    for c in range(NCORES):
        b, qoff = c // 2, (c % 2) * NQ
        oc = res.results[c]["out"].reshape(128, IT, DK).transpose(1, 0, 2)
        out[b, qoff:qoff + NQ] = oc.reshape(NQ, DK)
    # gamma/beta are ones/zeros in this problem's setup; apply on host if not.
    g = np.asarray(gamma, np.float32)
    bt = np.asarray(beta, np.float32)
    if not (np.all(g == 1.0) and np.all(bt == 0.0)):
        out = out * g + bt
    kernel._last_results = res
    return out
